# revision 20
# baseline (speedup 1.0000x reference)
"""Trainium2 Bass kernel for nn_AnswerSynthesisDecoder (8 NeuronCores).

Strategy:
  - GRU hidden state h (2048) is sharded 8 ways (256 per core). Each scan
    step does ONE 8-core AllGather carrying [h_new slice (256) ; partial
    attention pre-activation z (1024)] so the attention needs no second
    collective and no replicated w_a streaming.
  - All recurrent matvecs run W-stationary on TensorE in bf16.
  - Everything off the recurrence critical path (w_r/u_r/v_r projection,
    maxout, vocab projection w_o, softmax) is deferred and batched (F=64)
    after the scan; vocab is sharded 8 ways (4000 rows/core).
"""
import os
import numpy as np

# Model dims (hardcoded per problem spec)
H = 512
D2 = 1024          # 2H
D4 = 2048          # 4H
V = 32000
L = 64
NC = 8             # cores
S = D4 // NC       # 256: h-slice per core
GR = 3 * S         # 768: gate rows per core (r/z/n x 256)
VS = V // NC       # 4000: vocab shard
RS = H // NC       # 64:  rm-slice rows per core
NSTEPS = int(os.environ.get("KERNEL_NSTEPS", L))

_CACHE = {}


def _build(nsteps):
    from concourse import bass, bacc, tile, mybir

    f32 = mybir.dt.float32
    bf16 = mybir.dt.bfloat16
    Tanh = mybir.ActivationFunctionType.Tanh
    Sigmoid = mybir.ActivationFunctionType.Sigmoid
    Exp = mybir.ActivationFunctionType.Exp
    ADD = mybir.AluOpType.add
    MULT = mybir.AluOpType.mult
    AX = mybir.AxisListType.X
    AC = mybir.AxisListType.C

    nc = bacc.Bacc("TRN2", target_bir_lowering=False, debug=False,
                   enable_asserts=False, num_devices=NC)

    # ---- External inputs (per-core shards; same shapes on all cores) ----
    def din(name, shape, dt=bf16):
        return nc.dram_tensor(name, list(shape), dt, kind="ExternalInput")

    m1t_d = din("m1t", (D2, GR))            # gru_w_ih[:,1024:2048][rows].T
    whht_d = din("whht", (D4, GR))          # gru_w_hh[rows].T
    wihwt_d = din("wihwt", (D2, GR))        # gru_w_ih[:,0:1024][rows].T
    bih_d = din("bih", (128, 6), f32)
    bhh_d = din("bhh", (128, 6), f32)
    wakt_d = din("wakt", (S, D2))           # w_a[:, colslice].T
    uat_d = din("uat", (D2, D2))            # u_a.T (replicated)
    vt_d = din("vt", (128, 8), f32)         # v (1024) as (128,8)
    wdt_d = din("wdt", (D2, S))             # w_d[rows_d0].T
    bd0_d = din("bd0", (128, 2), f32)
    xd0_d = din("xd0", (128, 8), f32)       # h_q[0] or h_p[0] as (128,8)
    urt_d = din("urt", (D2, 128))           # u_r[rrows].T
    vrt_d = din("vrt", (D4, 128))           # v_r[rrows].T
    wrt_d = din("wrt", (D2, 128))           # w_r[rrows].T
    wot_d = din("wot", (H, VS))             # w_o[vocab slice].T
    hps_d = din("hps", (512, D2), f32)      # h_p row-slice (4096/8)
    hq_d = din("hq", (32, D2), f32)         # full h_q
    anst_d = din("anst", (D2, L))           # ans_embeds.T (bf16)
    ones128_d = din("ones128", (128, 1), f32)
    onesrow_d = din("onesrow", (1, 128), f32)
    onessq_d = din("onessq", (128, 128), bf16)
    ones32_d = din("ones32", (32, 1), f32)  # value 1/8

    out_d = nc.dram_tensor("out", [L, VS], f32, kind="ExternalOutput")

    with tile.TileContext(nc) as tc:
        with (
            tc.tile_pool(name="wp", bufs=1) as wp,          # persistent SBUF
            tc.tile_pool(name="sp", bufs=3) as sp,          # per-step small tiles
            tc.tile_pool(name="dram", bufs=1, space="DRAM") as dp,
        ):
            # ================= SBUF weight tiles + loads =================
            m1t = wp.tile([128, 8, GR], bf16, name="m1t")
            whht = wp.tile([128, 16, GR], bf16, name="whht")
            wihwt = wp.tile([128, 8, GR], bf16, name="wihwt")
            wakt = wp.tile([128, 2, D2], bf16, name="wakt")
            uat = wp.tile([128, 8, D2], bf16, name="uat")
            wdt = wp.tile([128, 8, S], bf16, name="wdt")
            urt = wp.tile([128, 8, 128], bf16, name="urt")
            vrt = wp.tile([128, 16, 128], bf16, name="vrt")
            wrt = wp.tile([128, 8, 128], bf16, name="wrt")
            wot = wp.tile([128, 4, VS], bf16, name="wot")
            anst = wp.tile([128, 8, L], bf16, name="anst")
            hps = wp.tile([128, 4, D2], f32, name="hps")
            hq = wp.tile([32, D2], f32, name="hq")
            vt = wp.tile([128, 8], f32, name="vt")
            bih = wp.tile([128, 6], f32, name="bih")
            bhh = wp.tile([128, 6], f32, name="bhh")
            bd0 = wp.tile([128, 2], f32, name="bd0")
            xd0 = wp.tile([128, 8], f32, name="xd0")
            ones128 = wp.tile([128, 1], f32, name="ones128")
            onesrow = wp.tile([1, 128], f32, name="onesrow")
            onessq = wp.tile([128, 128], bf16, name="onessq")
            ones32 = wp.tile([32, 1], f32, name="ones32")

            def load3(sb, dr):
                # dram (kc_n*128, cols) -> sbuf (128, kc_n, cols)
                nc.sync.dma_start(
                    sb[:], dr.ap().rearrange("(kc p) c -> p kc c", p=128))

            # scan-critical first; epilogue-only weights last
            nc.sync.dma_start(xd0[:], xd0_d[:, :])
            nc.sync.dma_start(bd0[:], bd0_d[:, :])
            load3(wdt, wdt_d)
            load3(wakt, wakt_d)
            load3(hps, hps_d)
            nc.sync.dma_start(hq[:], hq_d[:, :])
            nc.sync.dma_start(vt[:], vt_d[:, :])
            nc.sync.dma_start(bih[:], bih_d[:, :])
            nc.sync.dma_start(bhh[:], bhh_d[:, :])
            nc.sync.dma_start(ones128[:], ones128_d[:, :])
            nc.sync.dma_start(ones32[:], ones32_d[:, :])
            nc.sync.dma_start(onesrow[:], onesrow_d[:, :])
            nc.sync.dma_start(onessq[:], onessq_d[:, :])
            load3(uat, uat_d)
            load3(anst, anst_d)
            load3(m1t, m1t_d)
            load3(whht, whht_d)
            load3(wihwt, wihwt_d)
            load3(wrt, wrt_d)
            load3(urt, urt_d)
            load3(vrt, vrt_d)
            load3(wot, wot_d)

            # ============= persistent state / scratch =============
            Hbf = wp.tile([128, L + 1, 16], bf16, name="Hbf")
            Cbf = wp.tile([128, L, 8], bf16, name="Cbf")
            Gi = wp.tile([128, 6, L], f32, name="Gi")
            uah = wp.tile([128, 8], f32, name="uah")
            Ar = wp.tile([128, L], f32, name="Ar")
            expT = wp.tile([64, VS], f32, name="expT")
            rmf = wp.tile([128, 4, L], f32, name="rmf")
            rmb = wp.tile([128, 4, L], bf16, name="rmb")
            probsT = wp.tile([64, VS], f32, name="probsT")

            # ============= collective bounce buffers =============
            PAY = S + D2  # 1280
            agin = dp.tile([L + 1, PAY], f32, name="agin")
            agouts = [
                dp.tile([NC, PAY], f32, name=f"agout{t}", addr_space="Shared")
                for t in range(nsteps + 1)
            ]
            ag0in = dp.tile([1, D2], f32, name="ag0in")
            ag0out = dp.tile([NC, D2], f32, name="ag0out", addr_space="Shared")
            agrm_in = dp.tile([RS, L], f32, name="agrm_in")
            agrm_out = dp.tile([NC, RS, L], f32, name="agrm_out",
                               addr_space="Shared")
            agss_in = dp.tile([1, L], f32, name="agss_in")
            agss_out = dp.tile([NC, L], f32, name="agss_out",
                               addr_space="Shared")

            RG = [list(range(NC))]

            def allgather(inap, outap):
                nc.gpsimd.collective_compute(
                    "AllGather", mybir.AluOpType.bypass,
                    replica_groups=RG, ins=[inap.opt()], outs=[outap.opt()])

            # ===================== PROLOGUE =====================
            with tc.tile_pool(name="pps", space="PSUM", bufs=1) as pps:
                # --- partial h sum (transposed): hsumT[p, q] =
                #     (1/8)*sum(h_q)[q*128+p] + sum(h_p slice)[q*128+p]
                hsum_ps = pps.tile([128, 8], f32, name="hsum_ps")
                for q in range(8):
                    for kc in range(4):
                        nc.tensor.matmul(
                            hsum_ps[:, q:q + 1],
                            hps[:, kc, q * 128:(q + 1) * 128], ones128[:],
                            start=(kc == 0), stop=False)
                    nc.tensor.matmul(
                        hsum_ps[:, q:q + 1],
                        hq[:, q * 128:(q + 1) * 128], ones32[:],
                        start=False, stop=True)
                hsum_sb = sp.tile([128, 8], f32, name="hsum_sb")
                nc.vector.tensor_copy(hsum_sb[:], hsum_ps[:])
                # payload word order: p-major (p*8 + q)
                nc.scalar.dma_start(
                    ag0in[0:1, :].rearrange("o (p q) -> (o p) q", q=8),
                    hsum_sb[:])
                allgather(ag0in[:, :], ag0out[:, :])

                # --- d0 = tanh(W_d[rows] @ xd0 + b)
                xd0b = sp.tile([128, 8], bf16, name="xd0b")
                nc.vector.tensor_copy(xd0b[:], xd0[:])
                d0_ps = pps.tile([128, 2], f32, name="d0_ps")
                for pc in range(2):
                    for kc in range(8):
                        nc.tensor.matmul(
                            d0_ps[:, pc:pc + 1],
                            wdt[:, kc, pc * 128:(pc + 1) * 128],
                            xd0b[:, kc:kc + 1],
                            start=(kc == 0), stop=(kc == 7))
                d0p = sp.tile([128, 2], f32, name="d0p")
                nc.vector.tensor_add(d0p[:], d0_ps[:], bd0[:])
                h_own = sp.tile([128, 2], f32, name="h_own", tag="h_own")
                nc.scalar.activation(h_own[:], d0p[:], Tanh)

                # --- Gi = W_ih_w[rows] @ ansT + b_ih  (128, 6, 64)
                gi_pro = pps.tile([128, 6, L], f32, name="gi_pro")
                for pc in range(6):
                    for kc in range(8):
                        nc.tensor.matmul(
                            gi_pro[:, pc, :],
                            wihwt[:, kc, pc * 128:(pc + 1) * 128],
                            anst[:, kc, :],
                            start=(kc == 0), stop=(kc == 7))
                for pc in range(6):
                    nc.vector.tensor_scalar_add(
                        Gi[:, pc, :], gi_pro[:, pc, :], bih[:, pc:pc + 1])

                # --- Ar = w_r[rrows] @ ansT  (128, 64)
                ar_ps = pps.tile([128, L], f32, name="ar_ps")
                for kc in range(8):
                    nc.tensor.matmul(ar_ps[:], wrt[:, kc, :], anst[:, kc, :],
                                     start=(kc == 0), stop=(kc == 7))
                nc.vector.tensor_copy(Ar[:], ar_ps[:])

                # --- h_glob from AG0; u_a_h = u_a @ h_glob
                hparts = sp.tile([128, NC, 8], f32, name="hparts")
                nc.scalar.dma_start(
                    hparts[:], ag0out[:, :].rearrange("r (p q) -> p r q", p=128))
                hg = sp.tile([128, 8], f32, name="hg")
                nc.vector.tensor_reduce(
                    hg[:], hparts[:].rearrange("p r q -> p q r"), AX, ADD)
                hgb = sp.tile([128, 8], bf16, name="hgb")
                nc.vector.tensor_copy(hgb[:], hg[:])
                uah_ps = pps.tile([128, 8], f32, name="uah_ps")
                for pc in range(8):
                    for kc in range(8):
                        nc.tensor.matmul(
                            uah_ps[:, pc:pc + 1],
                            uat[:, kc, pc * 128:(pc + 1) * 128],
                            hgb[:, kc:kc + 1],
                            start=(kc == 0), stop=(kc == 7))
                nc.vector.tensor_copy(uah[:], uah_ps[:])

            # ===================== SCAN =====================
            with tc.tile_pool(name="scps", space="PSUM", bufs=2) as scps:
                def send_state(t, pay_tile):
                    """z-partial from h slice (pay[:,0:2]); one DMA -> agin[t]."""
                    hnb = sp.tile([128, 2], bf16, name="hnb", tag="hnb")
                    nc.vector.tensor_copy(hnb[:], pay_tile[:, 0:2])
                    zp_ps = scps.tile([128, 8], f32, name="zp_ps", tag="zp_ps")
                    for pc in range(8):
                        for kc in range(2):
                            nc.tensor.matmul(
                                zp_ps[:, pc:pc + 1],
                                wakt[:, kc, pc * 128:(pc + 1) * 128],
                                hnb[:, kc:kc + 1],
                                start=(kc == 0), stop=(kc == 1))
                    nc.vector.tensor_copy(pay_tile[:, 2:10], zp_ps[:])
                    pj = agin[t:t + 1, :].rearrange("o (p j) -> (o p) j", j=10)
                    nc.sync.dma_start(pj[:, :], pay_tile[:])

                pay = sp.tile([128, 10], f32, name="pay", tag="pay")
                nc.vector.tensor_copy(pay[:, 0:2], h_own[:])
                h_own = pay[:, 0:2]
                send_state(0, pay)

                for t in range(nsteps):
                    allgather(agin[t:t + 1, :], agouts[t][:, :])

                    # --- unpack: rx[p, r, j]; j: 0,1 = h(qq), 2..9 = z(q)
                    rx = sp.tile([128, NC, 10], f32, name="rx", tag="rx")
                    nc.scalar.dma_start(
                        rx[:], agouts[t][:, :].rearrange("r (p j) -> p r j", p=128))

                    # --- bf16 copy of h_t (q = 2r+qq order) for matmul rhs
                    nc.vector.tensor_copy(Hbf[:, t, :], rx[:, :, 0:2])

                    # --- Y = (M1/2) @ [hA | hB]   (starts right after rx)
                    y_ps = scps.tile([128, 6, 2], f32, name="y_ps", tag="y_ps")
                    for pc in range(6):
                        for kc in range(8):
                            nc.tensor.matmul(
                                y_ps[:, pc, :],
                                m1t[:, kc, pc * 128:(pc + 1) * 128],
                                Hbf[:, t, kc:kc + 9:8],
                                start=(kc == 0), stop=(kc == 7))

                    # --- gh = W_hh[rows] @ h_t  (emit early: only needs Hbf)
                    gh_ps = scps.tile([128, 6], f32, name="gh_ps", tag="gh_ps")
                    for pc in range(6):
                        for kc in range(16):
                            nc.tensor.matmul(
                                gh_ps[:, pc:pc + 1],
                                whht[:, kc, pc * 128:(pc + 1) * 128],
                                Hbf[:, t, kc:kc + 1],
                                start=(kc == 0), stop=(kc == 15))

                    # --- attention: z sums, tanh, dot v, y = tanh((s0-s1)/2)
                    z0 = sp.tile([128, 8], f32, name="z0", tag="z0")
                    z1 = sp.tile([128, 8], f32, name="z1", tag="z1")
                    nc.vector.tensor_reduce(
                        z0[:], rx[:, 0:4, 2:10].rearrange("p r q -> p q r"),
                        AX, ADD)
                    nc.vector.tensor_reduce(
                        z1[:], rx[:, 4:8, 2:10].rearrange("p r q -> p q r"),
                        AX, ADD)
                    nc.vector.tensor_add(z0[:], z0[:], uah[:])
                    nc.vector.tensor_add(z1[:], z1[:], uah[:])
                    tz0 = sp.tile([128, 8], f32, name="tz0", tag="tz0")
                    tz1 = sp.tile([128, 8], f32, name="tz1", tag="tz1")
                    nc.scalar.activation(tz0[:], z0[:], Tanh)
                    nc.scalar.activation(tz1[:], z1[:], Tanh)
                    vm0 = sp.tile([128, 8], f32, name="vm0", tag="vm0")
                    vm1 = sp.tile([128, 8], f32, name="vm1", tag="vm1")
                    spair = sp.tile([128, 2], f32, name="spair", tag="spair")
                    nc.vector.tensor_mul(vm0[:], tz0[:], vt[:])
                    nc.vector.tensor_mul(vm1[:], tz1[:], vt[:])
                    nc.vector.tensor_reduce(spair[:, 0:1], vm0[:], AX, ADD)
                    nc.vector.tensor_reduce(spair[:, 1:2], vm1[:], AX, ADD)
                    dsp = sp.tile([128, 1], bf16, name="dsp", tag="dsp")
                    nc.vector.tensor_sub(dsp[:], spair[:, 0:1], spair[:, 1:2])
                    dsB_ps = scps.tile([128, 1], f32, name="dsB_ps",
                                       tag="dsB_ps", bufs=1)
                    nc.tensor.matmul(dsB_ps[:], onessq[:], dsp[:],
                                     start=True, stop=True)
                    yB = sp.tile([128, 1], f32, name="yB", tag="yB")
                    nc.scalar.activation(yB[:], dsB_ps[:], Tanh, scale=0.5)

                    # --- c2 = 2c = (hA+hB) + y*(hA-hB)  (weights pre-halved)
                    hs2 = sp.tile([128, 4, 2], f32, name="hs2", tag="hs2")
                    hd2 = sp.tile([128, 4, 2], f32, name="hd2", tag="hd2")
                    nc.vector.tensor_add(hs2[:], rx[:, 0:4, 0:2], rx[:, 4:8, 0:2])
                    nc.vector.tensor_sub(hd2[:], rx[:, 0:4, 0:2], rx[:, 4:8, 0:2])
                    cf = sp.tile([128, 4, 2], f32, name="cf", tag="cf")
                    nc.vector.scalar_tensor_tensor(
                        cf[:], hd2[:], yB[:, 0:1], hs2[:], MULT, ADD)
                    nc.vector.tensor_copy(Cbf[:, t, :], cf[:])

                    # --- gi = (Y0+Y1) + y*(Y0-Y1) + Gi[t]
                    ysb = sp.tile([128, 6, 2], f32, name="ysb", tag="ysb")
                    nc.vector.tensor_copy(ysb[:], y_ps[:])
                    yss = sp.tile([128, 6], f32, name="yss", tag="yss")
                    ydd = sp.tile([128, 6], f32, name="ydd", tag="ydd")
                    nc.vector.tensor_add(yss[:], ysb[:, :, 0], ysb[:, :, 1])
                    nc.vector.tensor_sub(ydd[:], ysb[:, :, 0], ysb[:, :, 1])
                    giA = sp.tile([128, 6], f32, name="giA", tag="giA")
                    nc.vector.scalar_tensor_tensor(
                        giA[:], ydd[:], yB[:, 0:1], yss[:], MULT, ADD)
                    gi = sp.tile([128, 6], f32, name="gi", tag="gi")
                    nc.vector.tensor_add(gi[:], giA[:], Gi[:, :, t])

                    # --- gates (all-tanh: sigmoid(x) = .5 + .5*tanh(.5x))
                    gh = sp.tile([128, 6], f32, name="gh", tag="gh")
                    nc.vector.tensor_add(gh[:], gh_ps[:], bhh[:])
                    rzp = sp.tile([128, 4], f32, name="rzp", tag="rzp")
                    nc.vector.tensor_add(rzp[:], gi[:, 0:4], gh[:, 0:4])
                    rz = sp.tile([128, 4], f32, name="rz", tag="rz")
                    nc.scalar.activation(rz[:], rzp[:], Tanh, scale=0.5)
                    # n = tanh(gi_n + .5*(gh_n + gh_n*rzt))
                    nr = sp.tile([128, 2], f32, name="nr", tag="nr")
                    nc.vector.tensor_mul(nr[:], rz[:, 0:2], gh[:, 4:6])
                    nr2 = sp.tile([128, 2], f32, name="nr2", tag="nr2")
                    nc.vector.tensor_add(nr2[:], nr[:], gh[:, 4:6])
                    npre = sp.tile([128, 2], f32, name="npre", tag="npre")
                    nc.vector.scalar_tensor_tensor(
                        npre[:], nr2[:], 0.5, gi[:, 4:6], MULT, ADD)
                    nn = sp.tile([128, 2], f32, name="nn", tag="nn")
                    nc.scalar.activation(nn[:], npre[:], Tanh)
                    # h' = n + (.5 + .5*zt)*(h - n)
                    dd = sp.tile([128, 2], f32, name="dd", tag="dd")
                    nc.vector.tensor_sub(dd[:], h_own[:], nn[:])
                    ee = sp.tile([128, 2], f32, name="ee", tag="ee")
                    nc.vector.tensor_mul(ee[:], dd[:], rz[:, 2:4])
                    ee2 = sp.tile([128, 2], f32, name="ee2", tag="ee2")
                    nc.vector.tensor_add(ee2[:], ee[:], dd[:])
                    pay = sp.tile([128, 10], f32, name="pay", tag="pay")
                    nc.vector.scalar_tensor_tensor(
                        pay[:, 0:2], ee2[:], 0.5, nn[:], MULT, ADD)
                    h_own = pay[:, 0:2]

                    send_state(t + 1, pay)

                # final gather of h_L (needed by v_r term of r_{L-1})
                allgather(agin[nsteps:nsteps + 1, :], agouts[nsteps][:, :])
                rxf = sp.tile([128, NC, 10], f32, name="rxf", tag="rx")
                nc.sync.dma_start(
                    rxf[:],
                    agouts[nsteps][:, :].rearrange("r (p j) -> p r j", p=128))
                nc.vector.tensor_copy(Hbf[:, nsteps, :], rxf[:, :, 0:2])

            # ===================== EPILOGUE =====================
            with tc.tile_pool(name="eps", space="PSUM", bufs=1) as eps:
                rlo_ps = eps.tile([64, L], f32, name="rlo_ps")
                rhi_ps = eps.tile([64, L], f32, name="rhi_ps")
                for half, rps_t in ((0, rlo_ps), (1, rhi_ps)):
                    for kc in range(8):
                        nc.tensor.matmul(
                            rps_t[:], urt[:, kc, half * 64:(half + 1) * 64],
                            Cbf[:, :, kc], start=(kc == 0), stop=False)
                    for kc in range(16):
                        nc.tensor.matmul(
                            rps_t[:], vrt[:, kc, half * 64:(half + 1) * 64],
                            Hbf[:, 1:L + 1, kc],
                            start=False, stop=(kc == 15))
                rlo = sp.tile([64, L], f32, name="rlo")
                rhi = sp.tile([64, L], f32, name="rhi")
                nc.vector.tensor_add(rlo[:], rlo_ps[:], Ar[0:64, :])
                nc.vector.tensor_add(rhi[:], rhi_ps[:], Ar[64:128, :])
                rm = sp.tile([64, L], f32, name="rm")
                nc.vector.tensor_max(rm[:], rlo[:], rhi[:])
                nc.sync.dma_start(agrm_in[:, :], rm[:])
                allgather(agrm_in[:, :], agrm_out[:, :, :])

                for kc in range(4):
                    for hh in range(2):
                        nc.sync.dma_start(
                            rmf[hh * 64:(hh + 1) * 64, kc, :],
                            agrm_out[2 * kc + hh:2 * kc + hh + 1, :, :].rearrange(
                                "o p t -> (o p) t"))
                nc.vector.tensor_copy(rmb[:], rmf[:])

                # logitsT (t on partitions) + exp + row sums
                FC = 500
                sacc = sp.tile([64, VS // FC], f32, name="sacc")
                for fc in range(VS // FC):
                    lt_ps = eps.tile([64, FC], f32, name="lt_ps", tag="lt_ps",
                                     bufs=2)
                    for kc in range(4):
                        nc.tensor.matmul(
                            lt_ps[:], rmb[:, kc, :],
                            wot[:, kc, fc * FC:(fc + 1) * FC],
                            start=(kc == 0), stop=(kc == 3))
                    nc.scalar.activation(
                        expT[:, fc * FC:(fc + 1) * FC], lt_ps[:], Exp,
                        accum_out=sacc[:, fc:fc + 1])
                ssum = sp.tile([64, 1], f32, name="ssum")
                nc.vector.tensor_reduce(ssum[:], sacc[:], AX, ADD)
                nc.sync.dma_start(agss_in[:, :], ssum[:])
                allgather(agss_in[:, :], agss_out[:, :])
                ssT = sp.tile([64, NC], f32, name="ssT")
                for r in range(NC):
                    nc.sync.dma_start(ssT[:, r:r + 1], agss_out[r:r + 1, :])
                stot = sp.tile([64, 1], f32, name="stot")
                nc.vector.tensor_reduce(stot[:], ssT[:], AX, ADD)
                sinv = sp.tile([64, 1], f32, name="sinv")
                nc.vector.reciprocal(sinv[:], stot[:])
                nc.vector.tensor_scalar_mul(probsT[:], expT[:], sinv[:, 0:1])
                nc.sync.dma_start(out_d[:, :], probsT[:])

    nc.compile()
    return nc


def _prep_in_maps(inputs):
    import ml_dtypes
    bf = ml_dtypes.bfloat16

    h_q = np.asarray(inputs["h_q"], np.float32)
    h_p = np.asarray(inputs["h_p"], np.float32)
    ans = np.asarray(inputs["ans_embeds"], np.float32)
    v_w = np.asarray(inputs["v_w"], np.float32)
    w_d_w = np.asarray(inputs["w_d_w"], np.float32)
    w_d_b = np.asarray(inputs["w_d_b"], np.float32)
    w_a_w = np.asarray(inputs["w_a_w"], np.float32)
    u_a_w = np.asarray(inputs["u_a_w"], np.float32)
    w_r_w = np.asarray(inputs["w_r_w"], np.float32)
    u_r_w = np.asarray(inputs["u_r_w"], np.float32)
    v_r_w = np.asarray(inputs["v_r_w"], np.float32)
    w_o_w = np.asarray(inputs["w_o_w"], np.float32)
    gwi = np.asarray(inputs["gru_w_ih"], np.float32)
    gwh = np.asarray(inputs["gru_w_hh"], np.float32)
    gbi = np.asarray(inputs["gru_b_ih"], np.float32)
    gbh = np.asarray(inputs["gru_b_hh"], np.float32)

    in_maps = []
    for c in range(NC):
        r0 = np.r_[S * c:S * (c + 1),
                   D4 + S * c:D4 + S * (c + 1),
                   2 * D4 + S * c:2 * D4 + S * (c + 1)]
        col0 = (S * c) % D2
        rr = np.r_[RS * c:RS * (c + 1), H + RS * c:H + RS * (c + 1)]
        m = {
            "m1t": np.ascontiguousarray(0.5 * gwi[r0, D2:D4].T).astype(bf),
            "whht": np.ascontiguousarray(gwh[r0, :].T).astype(bf),
            "wihwt": np.ascontiguousarray(gwi[r0, 0:D2].T).astype(bf),
            "bih": np.ascontiguousarray(gbi[r0].reshape(6, 128).T),
            "bhh": np.ascontiguousarray(gbh[r0].reshape(6, 128).T),
            "wakt": np.ascontiguousarray(w_a_w[:, col0:col0 + S].T).astype(bf),
            "uat": np.ascontiguousarray(u_a_w.T).astype(bf),
            "vt": np.ascontiguousarray(v_w[0].reshape(8, 128).T),
            "wdt": np.ascontiguousarray(w_d_w[col0:col0 + S, :].T).astype(bf),
            "bd0": np.ascontiguousarray(w_d_b[col0:col0 + S].reshape(2, 128).T),
            "xd0": np.ascontiguousarray(
                (h_q[0] if c < 4 else h_p[0]).reshape(8, 128).T),
            "urt": np.ascontiguousarray(0.5 * u_r_w[rr, :].T).astype(bf),
            "vrt": np.ascontiguousarray(v_r_w[rr, :].T).astype(bf),
            "wrt": np.ascontiguousarray(w_r_w[rr, :].T).astype(bf),
            "wot": np.ascontiguousarray(w_o_w[VS * c:VS * (c + 1), :].T).astype(bf),
            "hps": np.ascontiguousarray(h_p[512 * c:512 * (c + 1), :]),
            "hq": h_q,
            "anst": np.ascontiguousarray(ans.T).astype(bf),
            "ones128": np.ones((128, 1), np.float32),
            "onesrow": np.ones((1, 128), np.float32),
            "onessq": np.ones((128, 128), bf),
            "ones32": np.full((32, 1), 1.0 / NC, np.float32),
        }
        in_maps.append(m)
    return in_maps


def kernel(**inputs):
    from concourse import bass_utils
    if "nc" not in _CACHE:
        _CACHE["nc"] = _build(NSTEPS)
    nc = _CACHE["nc"]
    in_maps = _prep_in_maps(inputs)
    res = bass_utils.run_bass_kernel_spmd(nc, in_maps, core_ids=list(range(NC)))
    out = np.concatenate(
        [np.asarray(res.results[c]["out"], np.float32) for c in range(NC)],
        axis=1)
    return out


# revision 22
# speedup vs baseline: 1.0471x; 1.0471x over previous
"""Trainium2 Bass kernel for nn_AnswerSynthesisDecoder (8 NeuronCores).

Strategy:
  - GRU hidden state h (2048) is sharded 8 ways (256 per core). Each scan
    step does ONE 8-core AllGather carrying [h_new slice (256) ; partial
    attention pre-activation z (1024)] so the attention needs no second
    collective and no replicated w_a streaming.
  - All recurrent matvecs run W-stationary on TensorE in bf16.
  - Everything off the recurrence critical path (w_r/u_r/v_r projection,
    maxout, vocab projection w_o, softmax) is deferred and batched (F=64)
    after the scan; vocab is sharded 8 ways (4000 rows/core).
"""
import os
import numpy as np

# Model dims (hardcoded per problem spec)
H = 512
D2 = 1024          # 2H
D4 = 2048          # 4H
V = 32000
L = 64
NC = 8             # cores
S = D4 // NC       # 256: h-slice per core
GR = 3 * S         # 768: gate rows per core (r/z/n x 256)
VS = V // NC       # 4000: vocab shard
RS = H // NC       # 64:  rm-slice rows per core
NSTEPS = int(os.environ.get("KERNEL_NSTEPS", L))

_CACHE = {}


def _build(nsteps):
    from concourse import bass, bacc, tile, mybir

    f32 = mybir.dt.float32
    bf16 = mybir.dt.bfloat16
    Tanh = mybir.ActivationFunctionType.Tanh
    Sigmoid = mybir.ActivationFunctionType.Sigmoid
    Exp = mybir.ActivationFunctionType.Exp
    ADD = mybir.AluOpType.add
    MULT = mybir.AluOpType.mult
    AX = mybir.AxisListType.X
    AC = mybir.AxisListType.C

    nc = bacc.Bacc("TRN2", target_bir_lowering=False, debug=False,
                   enable_asserts=False, num_devices=NC)

    # ---- External inputs (per-core shards; same shapes on all cores) ----
    def din(name, shape, dt=bf16):
        return nc.dram_tensor(name, list(shape), dt, kind="ExternalInput")

    m1t_d = din("m1t", (D2, GR))            # gru_w_ih[:,1024:2048][rows].T
    whht_d = din("whht", (D4, GR))          # gru_w_hh[rows].T
    wihwt_d = din("wihwt", (D2, GR))        # gru_w_ih[:,0:1024][rows].T
    bih_d = din("bih", (128, 6), f32)
    bhh_d = din("bhh", (128, 6), f32)
    wakt_d = din("wakt", (S, D2))           # w_a[:, colslice].T
    uat_d = din("uat", (D2, D2))            # u_a.T (replicated)
    vt_d = din("vt", (128, 8), f32)         # v (1024) as (128,8)
    wdt_d = din("wdt", (D2, S))             # w_d[rows_d0].T
    bd0_d = din("bd0", (128, 2), f32)
    xd0_d = din("xd0", (128, 8), f32)       # h_q[0] or h_p[0] as (128,8)
    urt_d = din("urt", (D2, 128))           # u_r[rrows].T
    vrt_d = din("vrt", (D4, 128))           # v_r[rrows].T
    wrt_d = din("wrt", (D2, 128))           # w_r[rrows].T
    wot_d = din("wot", (H, VS))             # w_o[vocab slice].T
    hps_d = din("hps", (512, D2), f32)      # h_p row-slice (4096/8)
    hq_d = din("hq", (32, D2), f32)         # full h_q
    anst_d = din("anst", (D2, L))           # ans_embeds.T (bf16)
    ones128_d = din("ones128", (128, 1), f32)
    onesrow_d = din("onesrow", (1, 128), f32)
    onessq_d = din("onessq", (128, 128), bf16)
    ones32_d = din("ones32", (32, 1), f32)  # value 1/8

    out_d = nc.dram_tensor("out", [L, VS], f32, kind="ExternalOutput")

    with tile.TileContext(nc) as tc:
        with (
            tc.tile_pool(name="wp", bufs=1) as wp,          # persistent SBUF
            tc.tile_pool(name="sp", bufs=3) as sp,          # per-step small tiles
            tc.tile_pool(name="dram", bufs=1, space="DRAM") as dp,
        ):
            # ================= SBUF weight tiles + loads =================
            m1t = wp.tile([128, 8, GR], bf16, name="m1t")
            whht = wp.tile([128, 16, GR], bf16, name="whht")
            wihwt = wp.tile([128, 8, GR], bf16, name="wihwt")
            wakt = wp.tile([128, 2, D2], bf16, name="wakt")
            uat = wp.tile([128, 8, D2], bf16, name="uat")
            wdt = wp.tile([128, 8, S], bf16, name="wdt")
            urt = wp.tile([128, 8, 128], bf16, name="urt")
            vrt = wp.tile([128, 16, 128], bf16, name="vrt")
            wrt = wp.tile([128, 8, 128], bf16, name="wrt")
            wot = wp.tile([128, 4, VS], bf16, name="wot")
            anst = wp.tile([128, 8, L], bf16, name="anst")
            hps = wp.tile([128, 4, D2], f32, name="hps")
            hq = wp.tile([32, D2], f32, name="hq")
            vt = wp.tile([128, 8], f32, name="vt")
            bih = wp.tile([128, 6], f32, name="bih")
            bhh = wp.tile([128, 6], f32, name="bhh")
            bd0 = wp.tile([128, 2], f32, name="bd0")
            xd0 = wp.tile([128, 8], f32, name="xd0")
            ones128 = wp.tile([128, 1], f32, name="ones128")
            onesrow = wp.tile([1, 128], f32, name="onesrow")
            onessq = wp.tile([128, 128], bf16, name="onessq")
            ones32 = wp.tile([32, 1], f32, name="ones32")

            def load3(sb, dr):
                # dram (kc_n*128, cols) -> sbuf (128, kc_n, cols)
                nc.sync.dma_start(
                    sb[:], dr.ap().rearrange("(kc p) c -> p kc c", p=128))

            # scan-critical first; epilogue-only weights last
            nc.sync.dma_start(xd0[:], xd0_d[:, :])
            nc.sync.dma_start(bd0[:], bd0_d[:, :])
            load3(wdt, wdt_d)
            load3(wakt, wakt_d)
            load3(hps, hps_d)
            nc.sync.dma_start(hq[:], hq_d[:, :])
            nc.sync.dma_start(vt[:], vt_d[:, :])
            nc.sync.dma_start(bih[:], bih_d[:, :])
            nc.sync.dma_start(bhh[:], bhh_d[:, :])
            nc.sync.dma_start(ones128[:], ones128_d[:, :])
            nc.sync.dma_start(ones32[:], ones32_d[:, :])
            nc.sync.dma_start(onesrow[:], onesrow_d[:, :])
            nc.sync.dma_start(onessq[:], onessq_d[:, :])
            load3(uat, uat_d)
            load3(anst, anst_d)
            load3(m1t, m1t_d)
            load3(whht, whht_d)
            load3(wihwt, wihwt_d)
            load3(wrt, wrt_d)
            load3(urt, urt_d)
            load3(vrt, vrt_d)
            load3(wot, wot_d)

            # ============= persistent state / scratch =============
            Hbf = wp.tile([128, L + 1, 16], bf16, name="Hbf")
            Cbf = wp.tile([128, L, 8], bf16, name="Cbf")
            Gi = wp.tile([128, 6, L], f32, name="Gi")
            uah = wp.tile([128, 8], f32, name="uah")
            Ar = wp.tile([128, L], f32, name="Ar")
            expT = wp.tile([64, VS], f32, name="expT")
            rmf = wp.tile([128, 4, L], f32, name="rmf")
            rmb = wp.tile([128, 4, L], bf16, name="rmb")
            probsT = wp.tile([64, VS], f32, name="probsT")

            # ============= collective bounce buffers =============
            PAY = S + D2  # 1280
            agin = dp.tile([L + 1, PAY], f32, name="agin")
            agouts = [
                dp.tile([NC, PAY], f32, name=f"agout{t}", addr_space="Shared")
                for t in range(nsteps + 1)
            ]
            ag0in = dp.tile([1, D2], f32, name="ag0in")
            ag0out = dp.tile([NC, D2], f32, name="ag0out", addr_space="Shared")
            agrm_in = dp.tile([RS, L], f32, name="agrm_in")
            agrm_out = dp.tile([NC, RS, L], f32, name="agrm_out",
                               addr_space="Shared")
            agss_in = dp.tile([1, L], f32, name="agss_in")
            agss_out = dp.tile([NC, L], f32, name="agss_out",
                               addr_space="Shared")

            RG = [list(range(NC))]

            def allgather(inap, outap):
                nc.gpsimd.collective_compute(
                    "AllGather", mybir.AluOpType.bypass,
                    replica_groups=RG, ins=[inap.opt()], outs=[outap.opt()])

            # ===================== PROLOGUE =====================
            with tc.tile_pool(name="pps", space="PSUM", bufs=1) as pps:
                # --- partial h sum (transposed): hsumT[p, q] =
                #     (1/8)*sum(h_q)[q*128+p] + sum(h_p slice)[q*128+p]
                hsum_ps = pps.tile([128, 8], f32, name="hsum_ps")
                for q in range(8):
                    for kc in range(4):
                        nc.tensor.matmul(
                            hsum_ps[:, q:q + 1],
                            hps[:, kc, q * 128:(q + 1) * 128], ones128[:],
                            start=(kc == 0), stop=False)
                    nc.tensor.matmul(
                        hsum_ps[:, q:q + 1],
                        hq[:, q * 128:(q + 1) * 128], ones32[:],
                        start=False, stop=True)
                hsum_sb = sp.tile([128, 8], f32, name="hsum_sb")
                nc.vector.tensor_copy(hsum_sb[:], hsum_ps[:])
                # payload word order: p-major (p*8 + q)
                nc.scalar.dma_start(
                    ag0in[0:1, :].rearrange("o (p q) -> (o p) q", q=8),
                    hsum_sb[:])
                allgather(ag0in[:, :], ag0out[:, :])

                # --- d0 = tanh(W_d[rows] @ xd0 + b)
                xd0b = sp.tile([128, 8], bf16, name="xd0b")
                nc.vector.tensor_copy(xd0b[:], xd0[:])
                d0_ps = pps.tile([128, 2], f32, name="d0_ps")
                for pc in range(2):
                    for kc in range(8):
                        nc.tensor.matmul(
                            d0_ps[:, pc:pc + 1],
                            wdt[:, kc, pc * 128:(pc + 1) * 128],
                            xd0b[:, kc:kc + 1],
                            start=(kc == 0), stop=(kc == 7))
                d0p = sp.tile([128, 2], f32, name="d0p")
                nc.vector.tensor_add(d0p[:], d0_ps[:], bd0[:])
                h_own = sp.tile([128, 2], f32, name="h_own", tag="h_own")
                nc.scalar.activation(h_own[:], d0p[:], Tanh)

                # --- Gi = W_ih_w[rows] @ ansT + b_ih  (128, 6, 64)
                gi_pro = pps.tile([128, 6, L], f32, name="gi_pro")
                for pc in range(6):
                    for kc in range(8):
                        nc.tensor.matmul(
                            gi_pro[:, pc, :],
                            wihwt[:, kc, pc * 128:(pc + 1) * 128],
                            anst[:, kc, :],
                            start=(kc == 0), stop=(kc == 7))
                for pc in range(6):
                    nc.vector.tensor_scalar_add(
                        Gi[:, pc, :], gi_pro[:, pc, :], bih[:, pc:pc + 1])

                # --- Ar = w_r[rrows] @ ansT  (128, 64)
                ar_ps = pps.tile([128, L], f32, name="ar_ps")
                for kc in range(8):
                    nc.tensor.matmul(ar_ps[:], wrt[:, kc, :], anst[:, kc, :],
                                     start=(kc == 0), stop=(kc == 7))
                nc.vector.tensor_copy(Ar[:], ar_ps[:])

                # --- h_glob from AG0; u_a_h = u_a @ h_glob
                hparts = sp.tile([128, NC, 8], f32, name="hparts")
                nc.scalar.dma_start(
                    hparts[:], ag0out[:, :].rearrange("r (p q) -> p r q", p=128))
                hg = sp.tile([128, 8], f32, name="hg")
                nc.vector.tensor_reduce(
                    hg[:], hparts[:].rearrange("p r q -> p q r"), AX, ADD)
                hgb = sp.tile([128, 8], bf16, name="hgb")
                nc.vector.tensor_copy(hgb[:], hg[:])
                uah_ps = pps.tile([128, 8], f32, name="uah_ps")
                for pc in range(8):
                    for kc in range(8):
                        nc.tensor.matmul(
                            uah_ps[:, pc:pc + 1],
                            uat[:, kc, pc * 128:(pc + 1) * 128],
                            hgb[:, kc:kc + 1],
                            start=(kc == 0), stop=(kc == 7))
                nc.vector.tensor_copy(uah[:], uah_ps[:])

            # ===================== SCAN =====================
            with tc.tile_pool(name="scps", space="PSUM", bufs=2) as scps:
                def send_state(t, pay_tile):
                    """z-partial from h slice (pay[:,0:2]); one DMA -> agin[t]."""
                    hnb = sp.tile([128, 2], bf16, name="hnb", tag="hnb")
                    nc.vector.tensor_copy(hnb[:], pay_tile[:, 0:2])
                    zp_ps = scps.tile([128, 8], f32, name="zp_ps", tag="zp_ps")
                    for pc in range(8):
                        for kc in range(2):
                            nc.tensor.matmul(
                                zp_ps[:, pc:pc + 1],
                                wakt[:, kc, pc * 128:(pc + 1) * 128],
                                hnb[:, kc:kc + 1],
                                start=(kc == 0), stop=(kc == 1))
                    nc.vector.tensor_copy(pay_tile[:, 2:10], zp_ps[:])
                    pj = agin[t:t + 1, :].rearrange("o (p j) -> (o p) j", j=10)
                    nc.sync.dma_start(pj[:, :], pay_tile[:])

                pay = sp.tile([128, 10], f32, name="pay", tag="pay")
                nc.vector.tensor_copy(pay[:, 0:2], h_own[:])
                h_own = pay[:, 0:2]
                send_state(0, pay)

                for t in range(nsteps):
                    allgather(agin[t:t + 1, :], agouts[t][:, :])

                    # --- unpack: rx[p, r, j]; j: 0,1 = h(qq), 2..9 = z(q)
                    rx = sp.tile([128, NC, 10], f32, name="rx", tag="rx")
                    nc.scalar.dma_start(
                        rx[:], agouts[t][:, :].rearrange("r (p j) -> p r j", p=128))

                    # --- bf16 copy of h_t (q = 2r+qq order) for matmul rhs
                    nc.vector.tensor_copy(Hbf[:, t, :], rx[:, :, 0:2])

                    # --- Y = (M1/2) @ [hA | hB]   (starts right after rx)
                    y_ps = scps.tile([128, 6, 2], f32, name="y_ps", tag="y_ps")
                    for pc in range(6):
                        for kc in range(8):
                            nc.tensor.matmul(
                                y_ps[:, pc, :],
                                m1t[:, kc, pc * 128:(pc + 1) * 128],
                                Hbf[:, t, kc:kc + 9:8],
                                start=(kc == 0), stop=(kc == 7))

                    # --- attention: z sums, tanh, dsp = sum(v*(tz0-tz1))
                    z0 = sp.tile([128, 8], f32, name="z0", tag="z0")
                    z1 = sp.tile([128, 8], f32, name="z1", tag="z1")
                    nc.vector.tensor_reduce(
                        z0[:], rx[:, 0:4, 2:10].rearrange("p r q -> p q r"),
                        AX, ADD)
                    nc.vector.tensor_reduce(
                        z1[:], rx[:, 4:8, 2:10].rearrange("p r q -> p q r"),
                        AX, ADD)
                    nc.vector.tensor_add(z0[:], z0[:], uah[:])
                    nc.vector.tensor_add(z1[:], z1[:], uah[:])
                    tz0 = sp.tile([128, 8], f32, name="tz0", tag="tz0")
                    tz1 = sp.tile([128, 8], f32, name="tz1", tag="tz1")
                    nc.scalar.activation(tz0[:], z0[:], Tanh)
                    nc.scalar.activation(tz1[:], z1[:], Tanh)
                    tdz = sp.tile([128, 8], f32, name="tdz", tag="tdz")
                    nc.vector.tensor_sub(tdz[:], tz0[:], tz1[:])
                    vm0 = sp.tile([128, 8], f32, name="vm0", tag="vm0")
                    nc.vector.tensor_mul(vm0[:], tdz[:], vt[:])
                    dsp = sp.tile([128, 1], bf16, name="dsp", tag="dsp")
                    with nc.allow_low_precision(reason="8-elem dot, bf16 ok"):
                        nc.vector.tensor_reduce(dsp[:], vm0[:], AX, ADD)
                    dsB_ps = scps.tile([128, 1], f32, name="dsB_ps",
                                       tag="dsB_ps", bufs=1)
                    nc.tensor.matmul(dsB_ps[:], onessq[:], dsp[:],
                                     start=True, stop=True)
                    yB = sp.tile([128, 1], f32, name="yB", tag="yB")
                    nc.scalar.activation(yB[:], dsB_ps[:], Tanh, scale=0.5)

                    # --- gh = W_hh[rows] @ h_t  (emit early: only needs Hbf)
                    gh_ps = scps.tile([128, 6], f32, name="gh_ps", tag="gh_ps")
                    for pc in range(6):
                        for kc in range(16):
                            nc.tensor.matmul(
                                gh_ps[:, pc:pc + 1],
                                whht[:, kc, pc * 128:(pc + 1) * 128],
                                Hbf[:, t, kc:kc + 1],
                                start=(kc == 0), stop=(kc == 15))


                    # --- c2 = 2c = (hA+hB) + y*(hA-hB)  (weights pre-halved)
                    hs2 = sp.tile([128, 4, 2], f32, name="hs2", tag="hs2")
                    hd2 = sp.tile([128, 4, 2], f32, name="hd2", tag="hd2")
                    nc.vector.tensor_add(hs2[:], rx[:, 0:4, 0:2], rx[:, 4:8, 0:2])
                    nc.vector.tensor_sub(hd2[:], rx[:, 0:4, 0:2], rx[:, 4:8, 0:2])
                    cf = sp.tile([128, 4, 2], f32, name="cf", tag="cf")
                    nc.vector.scalar_tensor_tensor(
                        cf[:], hd2[:], yB[:, 0:1], hs2[:], MULT, ADD)
                    nc.vector.tensor_copy(Cbf[:, t, :], cf[:])

                    # --- gi = (Y0+Y1) + y*(Y0-Y1) + Gi[t]
                    ysb = sp.tile([128, 6, 2], f32, name="ysb", tag="ysb")
                    nc.vector.tensor_copy(ysb[:], y_ps[:])
                    yss = sp.tile([128, 6], f32, name="yss", tag="yss")
                    ydd = sp.tile([128, 6], f32, name="ydd", tag="ydd")
                    nc.vector.tensor_add(yss[:], ysb[:, :, 0], ysb[:, :, 1])
                    nc.vector.tensor_sub(ydd[:], ysb[:, :, 0], ysb[:, :, 1])
                    giA = sp.tile([128, 6], f32, name="giA", tag="giA")
                    nc.vector.scalar_tensor_tensor(
                        giA[:], ydd[:], yB[:, 0:1], yss[:], MULT, ADD)
                    gi = sp.tile([128, 6], f32, name="gi", tag="gi")
                    nc.vector.tensor_add(gi[:], giA[:], Gi[:, :, t])

                    # --- gates (all-tanh: sigmoid(x) = .5 + .5*tanh(.5x))
                    gh = sp.tile([128, 6], f32, name="gh", tag="gh")
                    nc.vector.tensor_add(gh[:], gh_ps[:], bhh[:])
                    rzp = sp.tile([128, 4], f32, name="rzp", tag="rzp")
                    nc.vector.tensor_add(rzp[:], gi[:, 0:4], gh[:, 0:4])
                    rz = sp.tile([128, 4], f32, name="rz", tag="rz")
                    nc.scalar.activation(rz[:], rzp[:], Tanh, scale=0.5)
                    # n = tanh(gi_n + .5*(gh_n + gh_n*rzt))
                    nr = sp.tile([128, 2], f32, name="nr", tag="nr")
                    nc.vector.tensor_mul(nr[:], rz[:, 0:2], gh[:, 4:6])
                    nr2 = sp.tile([128, 2], f32, name="nr2", tag="nr2")
                    nc.vector.tensor_add(nr2[:], nr[:], gh[:, 4:6])
                    npre = sp.tile([128, 2], f32, name="npre", tag="npre")
                    nc.vector.scalar_tensor_tensor(
                        npre[:], nr2[:], 0.5, gi[:, 4:6], MULT, ADD)
                    nn = sp.tile([128, 2], f32, name="nn", tag="nn")
                    nc.scalar.activation(nn[:], npre[:], Tanh)
                    # h' = n + (.5 + .5*zt)*(h - n)
                    dd = sp.tile([128, 2], f32, name="dd", tag="dd")
                    nc.vector.tensor_sub(dd[:], h_own[:], nn[:])
                    ee = sp.tile([128, 2], f32, name="ee", tag="ee")
                    nc.vector.tensor_mul(ee[:], dd[:], rz[:, 2:4])
                    ee2 = sp.tile([128, 2], f32, name="ee2", tag="ee2")
                    nc.vector.tensor_add(ee2[:], ee[:], dd[:])
                    pay = sp.tile([128, 10], f32, name="pay", tag="pay")
                    nc.vector.scalar_tensor_tensor(
                        pay[:, 0:2], ee2[:], 0.5, nn[:], MULT, ADD)
                    h_own = pay[:, 0:2]

                    send_state(t + 1, pay)

                # final gather of h_L (needed by v_r term of r_{L-1})
                allgather(agin[nsteps:nsteps + 1, :], agouts[nsteps][:, :])
                rxf = sp.tile([128, NC, 10], f32, name="rxf", tag="rx")
                nc.sync.dma_start(
                    rxf[:],
                    agouts[nsteps][:, :].rearrange("r (p j) -> p r j", p=128))
                nc.vector.tensor_copy(Hbf[:, nsteps, :], rxf[:, :, 0:2])

            # ===================== EPILOGUE =====================
            with tc.tile_pool(name="eps", space="PSUM", bufs=1) as eps:
                rlo_ps = eps.tile([64, L], f32, name="rlo_ps")
                rhi_ps = eps.tile([64, L], f32, name="rhi_ps")
                for half, rps_t in ((0, rlo_ps), (1, rhi_ps)):
                    for kc in range(8):
                        nc.tensor.matmul(
                            rps_t[:], urt[:, kc, half * 64:(half + 1) * 64],
                            Cbf[:, :, kc], start=(kc == 0), stop=False)
                    for kc in range(16):
                        nc.tensor.matmul(
                            rps_t[:], vrt[:, kc, half * 64:(half + 1) * 64],
                            Hbf[:, 1:L + 1, kc],
                            start=False, stop=(kc == 15))
                rlo = sp.tile([64, L], f32, name="rlo")
                rhi = sp.tile([64, L], f32, name="rhi")
                nc.vector.tensor_add(rlo[:], rlo_ps[:], Ar[0:64, :])
                nc.vector.tensor_add(rhi[:], rhi_ps[:], Ar[64:128, :])
                rm = sp.tile([64, L], f32, name="rm")
                nc.vector.tensor_max(rm[:], rlo[:], rhi[:])
                nc.sync.dma_start(agrm_in[:, :], rm[:])
                allgather(agrm_in[:, :], agrm_out[:, :, :])

                for kc in range(4):
                    for hh in range(2):
                        nc.sync.dma_start(
                            rmf[hh * 64:(hh + 1) * 64, kc, :],
                            agrm_out[2 * kc + hh:2 * kc + hh + 1, :, :].rearrange(
                                "o p t -> (o p) t"))
                nc.vector.tensor_copy(rmb[:], rmf[:])

                # logitsT (t on partitions) + exp + row sums
                FC = 500
                sacc = sp.tile([64, VS // FC], f32, name="sacc")
                for fc in range(VS // FC):
                    lt_ps = eps.tile([64, FC], f32, name="lt_ps", tag="lt_ps",
                                     bufs=2)
                    for kc in range(4):
                        nc.tensor.matmul(
                            lt_ps[:], rmb[:, kc, :],
                            wot[:, kc, fc * FC:(fc + 1) * FC],
                            start=(kc == 0), stop=(kc == 3))
                    nc.scalar.activation(
                        expT[:, fc * FC:(fc + 1) * FC], lt_ps[:], Exp,
                        accum_out=sacc[:, fc:fc + 1])
                ssum = sp.tile([64, 1], f32, name="ssum")
                nc.vector.tensor_reduce(ssum[:], sacc[:], AX, ADD)
                nc.sync.dma_start(agss_in[:, :], ssum[:])
                allgather(agss_in[:, :], agss_out[:, :])
                ssT = sp.tile([64, NC], f32, name="ssT")
                for r in range(NC):
                    nc.sync.dma_start(ssT[:, r:r + 1], agss_out[r:r + 1, :])
                stot = sp.tile([64, 1], f32, name="stot")
                nc.vector.tensor_reduce(stot[:], ssT[:], AX, ADD)
                sinv = sp.tile([64, 1], f32, name="sinv")
                nc.vector.reciprocal(sinv[:], stot[:])
                nc.vector.tensor_scalar_mul(probsT[:], expT[:], sinv[:, 0:1])
                nc.sync.dma_start(out_d[:, :], probsT[:])

    nc.compile()
    return nc


def _prep_in_maps(inputs):
    import ml_dtypes
    bf = ml_dtypes.bfloat16

    h_q = np.asarray(inputs["h_q"], np.float32)
    h_p = np.asarray(inputs["h_p"], np.float32)
    ans = np.asarray(inputs["ans_embeds"], np.float32)
    v_w = np.asarray(inputs["v_w"], np.float32)
    w_d_w = np.asarray(inputs["w_d_w"], np.float32)
    w_d_b = np.asarray(inputs["w_d_b"], np.float32)
    w_a_w = np.asarray(inputs["w_a_w"], np.float32)
    u_a_w = np.asarray(inputs["u_a_w"], np.float32)
    w_r_w = np.asarray(inputs["w_r_w"], np.float32)
    u_r_w = np.asarray(inputs["u_r_w"], np.float32)
    v_r_w = np.asarray(inputs["v_r_w"], np.float32)
    w_o_w = np.asarray(inputs["w_o_w"], np.float32)
    gwi = np.asarray(inputs["gru_w_ih"], np.float32)
    gwh = np.asarray(inputs["gru_w_hh"], np.float32)
    gbi = np.asarray(inputs["gru_b_ih"], np.float32)
    gbh = np.asarray(inputs["gru_b_hh"], np.float32)

    in_maps = []
    for c in range(NC):
        r0 = np.r_[S * c:S * (c + 1),
                   D4 + S * c:D4 + S * (c + 1),
                   2 * D4 + S * c:2 * D4 + S * (c + 1)]
        col0 = (S * c) % D2
        rr = np.r_[RS * c:RS * (c + 1), H + RS * c:H + RS * (c + 1)]
        m = {
            "m1t": np.ascontiguousarray(0.5 * gwi[r0, D2:D4].T).astype(bf),
            "whht": np.ascontiguousarray(gwh[r0, :].T).astype(bf),
            "wihwt": np.ascontiguousarray(gwi[r0, 0:D2].T).astype(bf),
            "bih": np.ascontiguousarray(gbi[r0].reshape(6, 128).T),
            "bhh": np.ascontiguousarray(gbh[r0].reshape(6, 128).T),
            "wakt": np.ascontiguousarray(w_a_w[:, col0:col0 + S].T).astype(bf),
            "uat": np.ascontiguousarray(u_a_w.T).astype(bf),
            "vt": np.ascontiguousarray(v_w[0].reshape(8, 128).T),
            "wdt": np.ascontiguousarray(w_d_w[col0:col0 + S, :].T).astype(bf),
            "bd0": np.ascontiguousarray(w_d_b[col0:col0 + S].reshape(2, 128).T),
            "xd0": np.ascontiguousarray(
                (h_q[0] if c < 4 else h_p[0]).reshape(8, 128).T),
            "urt": np.ascontiguousarray(0.5 * u_r_w[rr, :].T).astype(bf),
            "vrt": np.ascontiguousarray(v_r_w[rr, :].T).astype(bf),
            "wrt": np.ascontiguousarray(w_r_w[rr, :].T).astype(bf),
            "wot": np.ascontiguousarray(w_o_w[VS * c:VS * (c + 1), :].T).astype(bf),
            "hps": np.ascontiguousarray(h_p[512 * c:512 * (c + 1), :]),
            "hq": h_q,
            "anst": np.ascontiguousarray(ans.T).astype(bf),
            "ones128": np.ones((128, 1), np.float32),
            "onesrow": np.ones((1, 128), np.float32),
            "onessq": np.ones((128, 128), bf),
            "ones32": np.full((32, 1), 1.0 / NC, np.float32),
        }
        in_maps.append(m)
    return in_maps


def kernel(**inputs):
    from concourse import bass_utils
    if "nc" not in _CACHE:
        _CACHE["nc"] = _build(NSTEPS)
    nc = _CACHE["nc"]
    in_maps = _prep_in_maps(inputs)
    res = bass_utils.run_bass_kernel_spmd(nc, in_maps, core_ids=list(range(NC)))
    out = np.concatenate(
        [np.asarray(res.results[c]["out"], np.float32) for c in range(NC)],
        axis=1)
    return out
